# revision 25
# baseline (speedup 1.0000x reference)
"""Decorrelation (ZCA-whitening) normalization kernel for Trainium2 (Bass/Tile).

Full input (64, 56, 56, 256) f32. Data-parallel over batch across 8 NeuronCores
(8 batches -> 25088 pixels per core). Per core:

  Pass 1: stream pixel-major (128px, 14, 256ch) f32 chunks from HBM, cast to
          fp16 (DVE), accumulate per-half 128x128 second-moment Gram blocks on
          the PE (PSUM f32), PE-transpose every (128px,128ch) tile to
          channel-major fp16 (8-unit PSUM banks), drain on ACT with accum_out
          giving the per-channel sums for free. Resident fp16 tiles: 12.8 MB.
  Stats:  extract only the block-diagonal (group,16,16) Gram entries + channel
          sums -> one 18KB AllReduce. Newton-Schulz on f32r-bitcast matmuls,
          restructured as A=P^2, B=P*Sh (Sh=-0.5*sigma_n), P'=1.5P+A@B with the
          combine done by one DVE scalar_tensor_tensor reading PSUM directly.
          No separate mean-subtract pass: mean folds into pass 2.
  Pass 2: per PSUM tile, rank-1 init ones x (-wm@mu) (K=1 matmul), then the
          whitening matmuls accumulate on top; drains alternate DVE/ACT to
          staging, DMA out.

HBM traffic per core = 1x read + 1x write (pass 2 reads nothing from HBM).
"""

import sys

import numpy as np

for _p in ("/root/.axon_site/_ro/trn_rl_repo", "/opt/trn_rl_repo"):
    if _p not in sys.path:
        sys.path.append(_p)

# ---------------------------------------------------------------- constants
B, W, H, C = 64, 56, 56, 256
N_CORES = 8
B_LOC = B // N_CORES                # 8 batches per core
N_LOC = B_LOC * W * H               # 25088 pixels per core
N_TOT = B * W * H                   # 200704 pixels total
P = 128                             # partitions
UJ = 14                             # pixel-tiles (units) per chunk
CPX = UJ * P                        # 1792 pixels per chunk
NCHUNK = N_LOC // CPX               # 14 chunks per core
EPS = 1e-3
ITER_NUM = 5
NGRP = 8                            # 16x16 groups per 128-ch half
SB = 68                             # stats block: 2x32 Gd blocks + 2 sums + pad

assert NCHUNK * CPX == N_LOC

_STATE = {}


def _build_nc(variant=()):
    import concourse.bacc as bacc
    import concourse.tile as tile
    from concourse import mybir
    from contextlib import ExitStack

    f32 = mybir.dt.float32
    f32r = mybir.dt.float32r
    f16 = mybir.dt.float16
    Alu = mybir.AluOpType
    Act = mybir.ActivationFunctionType
    Axis = mybir.AxisListType

    nc = bacc.Bacc("TRN2", target_bir_lowering=False, debug=False,
                   num_devices=N_CORES)

    x = nc.dram_tensor("x", [N_LOC, C], f32, kind="ExternalInput").ap()
    y = nc.dram_tensor("y", [N_LOC, C], f32, kind="ExternalOutput").ap()
    c_id16 = nc.dram_tensor("c_id16", [P, P], f16, kind="ExternalInput").ap()
    c_eye = nc.dram_tensor("c_eye", [P, P], f32, kind="ExternalInput").ap()
    c_epseye = nc.dram_tensor("c_epseye", [P, P], f32, kind="ExternalInput").ap()
    c_mask = nc.dram_tensor("c_mask", [P, P], f32, kind="ExternalInput").ap()

    ns_f32 = "nsf32" in variant

    with tile.TileContext(nc) as tc, ExitStack() as octx:
        # ---------------- long-lived pools
        consts = octx.enter_context(tc.tile_pool(name="consts", bufs=1))
        resp = octx.enter_context(tc.tile_pool(name="resident", bufs=1))
        statp = octx.enter_context(tc.tile_pool(name="stats", bufs=1))

        id16 = consts.tile([P, P], f16, name="id16")
        eye = consts.tile([P, P], f32, name="eye")
        epseye = consts.tile([P, P], f32, name="epseye")
        mask = consts.tile([P, P], f32, name="mask")

        # stats block for AllReduce: [Gd_a(32) | Gd_b(32) | s_a | s_b | pad]
        statsb = statp.tile([P, SB], f32, name="statsb")
        use_p2p = "p2p" in variant
        if use_p2p:
            # manual all-reduce: every core XOR-broadcasts its stats into slot
            # k of peer (me^k)'s gather buffer; slot k thus holds core me^k.
            gbuf = statp.tile([P, 8, SB], f32, name="gbuf")
            red4 = statp.tile([P, 4, SB], f32, name="red4")
            red2 = statp.tile([P, 2, SB], f32, name="red2")
            rsem = nc.alloc_semaphore("p2p_arrive")
            lsem = nc.alloc_semaphore("p2p_sent")
        # per-drain accum_out columns: h0 -> cols 0..27, h1 -> cols 32..59
        acc_cols = statp.tile([P, 64], f32, name="acc_cols")

        # channel-major fp16 resident tiles: one per (chunk, half)
        res = [[resp.tile([P, UJ, P], f16, name=f"res_{c}_{h}")
                for h in range(2)] for c in range(NCHUNK)]

        xv = x.rearrange("(c j p) ch -> c p j ch", p=P, j=UJ)
        yv = y.rearrange("(c j p) ch -> c p j ch", p=P, j=UJ)

        # ================= PASS 1 =================
        with ExitStack() as ctx:
            loadp = ctx.enter_context(tc.tile_pool(name="loadp", bufs=3))
            castp = ctx.enter_context(tc.tile_pool(name="castp", bufs=3))
            gps = ctx.enter_context(tc.tile_pool(name="gpsum", bufs=1, space="PSUM"))
            trps = ctx.enter_context(tc.tile_pool(name="trpsum", bufs=4, space="PSUM"))

            g_ps = [gps.tile([P, P], f32, name=f"G_{h}") for h in range(2)]

            for ci in range(NCHUNK):
                xt = loadp.tile([P, UJ, C], f32, name="xt")
                nc.gpsimd.dma_start(out=xt, in_=xv[ci])
                if ci == 0:
                    nc.gpsimd.dma_start(out=id16, in_=c_id16)
                    nc.gpsimd.dma_start(out=eye, in_=c_eye)
                    nc.gpsimd.dma_start(out=epseye, in_=c_epseye)
                    nc.gpsimd.dma_start(out=mask, in_=c_mask)
                    # pre-load the ACT sqrt table so the mid-section sqrt
                    # doesn't stall the bias pass behind a table load
                    sqwarm = statp.tile([1, 1], f32, name="sqwarm")
                    nc.scalar.activation(out=sqwarm, in_=eye[0:1, 0:1],
                                         func=Act.Sqrt)
                xh = castp.tile([P, UJ, C], f16, name="xh")
                nc.vector.tensor_copy(out=xh[:, 0:7, :], in_=xt[:, 0:7, :])
                nc.vector.tensor_copy(out=xh[:, 7:UJ, :], in_=xt[:, 7:UJ, :])

                # Gram accumulation (fp16 in, f32 PSUM): G_h += T_h^T @ T_h
                for j in range(UJ):
                    first = ci == 0 and j == 0
                    last = ci == NCHUNK - 1 and j == UJ - 1
                    for h in range(2):
                        sl = xh[:, j, h * P:(h + 1) * P]
                        nc.tensor.matmul(g_ps[h], sl, sl, start=first,
                                         stop=last, skip_group_check=True)

                # PE transpose -> channel-major fp16; ACT drain with accum_out
                # (per-channel sums come for free from the drains)
                for h in range(2):
                    for t, b0 in enumerate(range(0, UJ, 8)):
                        bn = min(8, UJ - b0)
                        tp = trps.tile([P, 8, P], f16, name="tp")
                        for k in range(bn):
                            nc.tensor.matmul(
                                tp[:, k, :], xh[:, b0 + k, h * P:(h + 1) * P],
                                id16, is_transpose=True, skip_group_check=True)
                        col = h * 32 + 2 * ci + t
                        nc.scalar.activation(
                            out=res[ci][h][:, b0:b0 + bn, :], in_=tp[:, :bn, :],
                            func=Act.Copy,
                            accum_out=acc_cols[:, col:col + 1])

            if use_p2p:
                # preps AFTER every input dma_start: prepare_only ring entries
                # must not sit ahead of untriggered direct DMAs (they block
                # the FIFO); source read is deferred to trigger_dma
                for k in range(1, 8):
                    rd = [None] * 8
                    rd[k] = (0, k)
                    nc.gpsimd.remote_dma_broadcast(
                        out_ap=gbuf[:, k, :], in_ap=statsb,
                        remote_sem=rsem, local_sem=lsem, rdests=rd)

            # tail: 32-aligned block-diagonal Gram extract + channel sums
            # (each 32x32 block holds two 16x16 group blocks + junk that the
            # mask multiply kills later)
            for h in range(2):
                for k in range(4):
                    src = g_ps[h][32 * k:32 * (k + 1), 32 * k:32 * (k + 1)]
                    dst = statsb[32 * k:32 * (k + 1), h * 32:(h + 1) * 32]
                    if k % 2 == 0:
                        nc.scalar.activation(out=dst, in_=src, func=Act.Copy)
                    else:
                        nc.vector.tensor_copy(out=dst, in_=src)
            for h in range(2):
                nc.vector.reduce_sum(out=statsb[:, 64 + h:65 + h],
                                     in_=acc_cols[:, h * 32:h * 32 + 2 * NCHUNK],
                                     axis=Axis.X)
            nc.vector.memset(statsb[:, 66:SB], 0.0)

        # ================= ALL-REDUCE =================
        with ExitStack() as ctx:
            arst = statp.tile([P, SB], f32, name="arst")
            if use_p2p:
                nc.vector.tensor_copy(out=gbuf[:, 0, :], in_=statsb)
                nc.gpsimd.trigger_dma(count=None)
                # gate + reduce stay on gpsimd AFTER the trigger in queue
                # order: at the sim-scheduled trigger time nothing downstream
                # of the gate can have run, so the sem-assigner samples safe
                # engine ticks (no runtime cycle). The arrival wait
                # (p2p_arrive >= 14: 7 peers x 2 engines) is attached to the
                # first add after tile scheduling.
                p2p_gate_inst = nc.gpsimd.tensor_tensor(
                    out=red4, in0=gbuf[:, 0:4, :], in1=gbuf[:, 4:8, :],
                    op=Alu.add)
                nc.gpsimd.tensor_tensor(out=red2, in0=red4[:, 0:2, :],
                                        in1=red4[:, 2:4, :], op=Alu.add)
                nc.gpsimd.tensor_tensor(out=arst, in0=red2[:, 0, :],
                                        in1=red2[:, 1, :], op=Alu.add)
            else:
                dramp = ctx.enter_context(
                    tc.tile_pool(name="dram", bufs=1, space="DRAM"))
                cc_in = dramp.tile([P, SB], f32, name="cc_in")
                cc_out = dramp.tile([P, SB], f32, name="cc_out")
                nc.gpsimd.dma_start(out=cc_in, in_=statsb)
                nc.gpsimd.collective_compute(
                    "AllReduce", mybir.AluOpType.add,
                    replica_groups=[list(range(N_CORES))],
                    ins=[cc_in.opt()], outs=[cc_out.opt()])
                nc.gpsimd.dma_start(out=arst, in_=cc_out)

            # ============= Newton-Schulz (both halves interleaved) =========
            nsp = ctx.enter_context(tc.tile_pool(name="nsp", bufs=10))
            nps = ctx.enter_context(tc.tile_pool(name="nspsum", bufs=3, space="PSUM"))
            npsS = ctx.enter_context(tc.tile_pool(name="nspsumS", bufs=2, space="PSUM"))

            fns = f32 if ns_f32 else f32r

            wm16 = [statp.tile([P, P], f16, name=f"wm16_{h}") for h in range(2)]
            nmu = [statp.tile([P, 1], f32, name=f"nmu_{h}") for h in range(2)]

            # -mean columns first (DVE), so the ACT bias-subtract pass can
            # start while Newton-Schulz runs (NS itself never touches ACT)
            for h in range(2):
                nc.vector.tensor_scalar_mul(out=nmu[h],
                                            in0=arst[:, 64 + h:65 + h],
                                            scalar1=-1.0 / N_TOT)

            # mu rows for both halves via one f32 PE transpose
            colpad = nsp.tile([P, P], f32, name="colpad", tag="nsbig")
            nc.vector.memset(colpad, 0.0)
            for h in range(2):
                nc.vector.tensor_scalar_mul(out=colpad[:, 32 * h:32 * h + 1],
                                            in0=arst[:, 64 + h:65 + h],
                                            scalar1=1.0 / N_TOT)
            rp_ps = nps.tile([P, P], f32, name="rp_ps", tag="nsps")
            nc.tensor.matmul(rp_ps, colpad, eye, is_transpose=True,
                             skip_group_check=True)
            rowpad = nsp.tile([P, P], f32, name="rowpad", tag="nsbig")
            nc.vector.tensor_copy(out=rowpad, in_=rp_ps)

            sig = []
            sh = []
            tvec = []
            for h in range(2):
                # sigma, scattered from the block-diagonal AllReduce payload
                # (scale (1-eps)/N folded into the scatter copies)
                sg = nsp.tile([P, P], f32, name=f"sig_{h}", tag="sig")
                nc.vector.memset(sg, 0.0)
                for k in range(4):
                    src = arst[32 * k:32 * (k + 1), h * 32:(h + 1) * 32]
                    dst = sg[32 * k:32 * (k + 1), 32 * k:32 * (k + 1)]
                    nc.vector.tensor_scalar_mul(out=dst, in0=src,
                                                scalar1=(1.0 - EPS) / N_TOT)

                # outer product mu_h mu_h^T via K=1 rank-1 matmul
                o_ps = nps.tile([P, P], f32, name="o_ps", tag="nsps")
                nc.tensor.matmul(o_ps, rowpad[32 * h:32 * h + 1, :],
                                 rowpad[32 * h:32 * h + 1, :],
                                 skip_group_check=True)
                osc = nsp.tile([P, P], f32, name="osc", tag="nsbig")
                nc.vector.tensor_scalar_mul(out=osc, in0=o_ps,
                                            scalar1=-(1.0 - EPS))
                # sig = (sig + osc) * mask + eps*I
                nc.vector.tensor_add(out=sg, in0=sg, in1=osc)
                nc.vector.tensor_mul(out=sg, in0=sg, in1=mask)
                nc.vector.tensor_add(out=sg, in0=sg, in1=epseye)
                sig.append(sg)

                # per-group trace, spread back to rows via mask matmul
                djunk = nsp.tile([P, P], f32, name="djunk", tag="nsbig")
                dcol = nsp.tile([P, 1], f32, name="dcol", tag="nssmall")
                nc.vector.tensor_mul(out=djunk, in0=sg, in1=eye)
                nc.vector.reduce_sum(out=dcol, in_=djunk, axis=Axis.X)
                tv_ps = npsS.tile([P, 1], f32, name="tv_ps", tag="nsps1")
                nc.tensor.matmul(tv_ps, mask, dcol, skip_group_check=True)

                # Sh = -0.5 * sigma / trace;  rs = rsqrt(trace) for later
                rinv = nsp.tile([P, 1], f32, name=f"rinv_{h}", tag="nssmall")
                nc.vector.reciprocal(out=rinv, in_=tv_ps)
                rs = nsp.tile([P, 1], f32, name=f"rs_{h}", tag="nssmall")
                nc.scalar.activation(out=rs, in_=rinv, func=Act.Sqrt)
                tvec.append(rs)
                rneg = nsp.tile([P, 1], f32, name="rneg", tag="nssmall")
                nc.vector.tensor_scalar_mul(out=rneg, in0=rinv, scalar1=-0.5)
                # psh = [P | Sh]: one 256-wide matmul yields [P^2 | P@Sh]
                psh = nsp.tile([P, 2 * P], fns, name=f"psh_{h}", tag="psh")
                nc.vector.tensor_scalar_mul(out=psh[:, P:2 * P], in0=sg,
                                            scalar1=rneg)
                sh.append(psh)

            # P1 = 1.5*I + Sh  (== iteration 1 with P0 = I)
            for h in range(2):
                nc.vector.scalar_tensor_tensor(
                    out=sh[h][:, 0:P], in0=eye, scalar=1.5,
                    in1=sh[h][:, P:2 * P], op0=Alu.mult, op1=Alu.add)

            # remaining ITER_NUM-1 iterations, halves interleaved:
            #   [A | B] = P^T @ [P | Sh] in one 256-wide (full-rate f32r)
            #   matmul, then P' = 1.5*P + A @ B
            for it in range(ITER_NUM - 1):
                ab_s = [None, None]
                for h in range(2):
                    ab_ps = nps.tile([P, 2 * P], f32, name="ab_ps", tag="nsabp")
                    nc.tensor.matmul(ab_ps, sh[h][:, 0:P], sh[h],
                                     skip_group_check=True)
                    ab_s[h] = nsp.tile([P, 2 * P], fns, name="ab_s", tag="nsab")
                    nc.vector.tensor_copy(out=ab_s[h], in_=ab_ps)
                for h in range(2):
                    c_ps = nps.tile([P, P], f32, name="c_ps", tag="nsps")
                    nc.tensor.matmul(c_ps, ab_s[h][:, 0:P], ab_s[h][:, P:2 * P],
                                     skip_group_check=True)
                    nc.vector.scalar_tensor_tensor(
                        out=sh[h][:, 0:P], in0=sh[h][:, 0:P], scalar=1.5,
                        in1=c_ps, op0=Alu.mult, op1=Alu.add)

            # wm = P * rsqrt(trace)
            for h in range(2):
                nc.vector.tensor_scalar_mul(out=wm16[h], in0=sh[h][:, 0:P],
                                            scalar1=tvec[h])

            # mean-subtract the resident tiles in place on ACT, overlapping
            # with the (ACT-free) Newton-Schulz above
            for ci in range(NCHUNK):
                for h in range(2):
                    nc.scalar.activation(out=res[ci][h], in_=res[ci][h],
                                         func=Act.Identity, bias=nmu[h])

        # ================= PASS 2 =================
        with ExitStack() as ctx:
            stagep = ctx.enter_context(tc.tile_pool(name="stagep", bufs=4))
            yps = ctx.enter_context(tc.tile_pool(name="ypsum", bufs=8, space="PSUM"))

            for ci in range(NCHUNK):
                st = stagep.tile([P, UJ, C], f32, name="st")
                # block order (t, h) so each half-chunk [j0:j0+8) completes
                # early and its DMA can start while the rest drains
                for t, b0 in enumerate(range(0, UJ, 4)):
                    bn = min(4, UJ - b0)
                    for h in range(2):
                        yp = yps.tile([P, 4, P], f32, name="yp")
                        for k in range(bn):
                            nc.tensor.matmul(yp[:, k, :],
                                             res[ci][h][:, b0 + k, :],
                                             wm16[h], skip_group_check=True)
                        dst = st[:, b0:b0 + bn, h * P:(h + 1) * P]
                        # ACT is busy with the bias pass for the first chunks
                        if ci < 8 or (t + h) % 2 == 0:
                            nc.vector.tensor_copy(out=dst, in_=yp[:, :bn, :])
                        else:
                            nc.scalar.activation(out=dst, in_=yp[:, :bn, :],
                                                 func=Act.Copy)
                    if t == 1:
                        nc.gpsimd.dma_start(out=yv[ci][:, 0:8, :],
                                            in_=st[:, 0:8, :])
                nc.gpsimd.dma_start(out=yv[ci][:, 8:UJ, :],
                                    in_=st[:, 8:UJ, :])

    if use_p2p:
        w = mybir.SyncWait(sync_type="semaphore", id=rsem.num,
                           wait_mode="sem-ge-imm", wait_value=14,
                           ant_name="p2p_arrive")
        inst = p2p_gate_inst.ins
        si = inst.sync_info
        if si is None:
            inst.sync_info = mybir.SyncInfo(on_wait=[w], on_update=[])
        else:
            inst.sync_info = mybir.SyncInfo(on_wait=list(si.on_wait) + [w],
                                            on_update=list(si.on_update))
    nc.compile()
    return nc


def _get_nc(variant=()):
    key = ("nc",) + tuple(sorted(variant))
    if key not in _STATE:
        _STATE[key] = _build_nc(variant)
    return _STATE[key]


def _consts():
    g16 = np.eye(P, dtype=np.float16)
    eye = np.eye(P, dtype=np.float32)
    epseye = (EPS * np.eye(P)).astype(np.float32)
    mask = np.zeros((P, P), dtype=np.float32)
    for g in range(P // 16):
        mask[g * 16:(g + 1) * 16, g * 16:(g + 1) * 16] = 1.0
    return {"c_id16": g16, "c_eye": eye, "c_epseye": epseye, "c_mask": mask}


def _run(x, trace=False, variant=()):
    from concourse.bass_utils import run_bass_kernel_spmd

    x = np.ascontiguousarray(x, dtype=np.float32).reshape(B, W * H * C)
    consts = _consts()
    in_maps = []
    for i in range(N_CORES):
        m = {"x": np.ascontiguousarray(
            x[i * B_LOC:(i + 1) * B_LOC].reshape(N_LOC, C))}
        m.update(consts)
        in_maps.append(m)

    nc = _get_nc(variant)
    r = run_bass_kernel_spmd(nc, in_maps, core_ids=list(range(N_CORES)),
                             trace=trace)
    out = np.concatenate([r.results[i]["y"].reshape(B_LOC, W, H, C)
                          for i in range(N_CORES)], axis=0)
    return out, r


def kernel(inputs):
    return _run(inputs, trace=False)[0]


if __name__ == "__main__":
    x = np.random.randn(B, W, H, C).astype(np.float32)
    out, _ = _run(x)
    print(out.shape, out.dtype)
